# revision 7
# baseline (speedup 1.0000x reference)
"""Trainium2 Bass kernel for dual-attention (DisKT-style) nn module.

Math per (batch, head) with S=1024, dk=64, all on-chip in [k, q] layout:
    sT       = (k_h @ q_h^T)            (+ -1e30 on causal-dead diag block)
    E1T      = exp(sT / 8)              (causally-dead region never computed)
    r1[q]    = sum_k E1T[k, q]          (ones^T @ E1T, PSUM broadcast rows)
    p1       = E1T * rec1[q]
    E2''     = exp(p1) - 1              <- bf16-friendly: small values keep
                                           precision; the "+1" of every key
                                           becomes an exact rank-1 vtot fixup
    outT     = (cm*vhi)^T @ E2'' + (cm*vlo)^T @ E2''   (cm pre-masked on host,
                                            v split hi+lo kills bf16 v error)
    r2       = 1024 + cmrep^T @ E2''
    out      = (outT + vtot) * (1/r2) ;  out[:, q=0] = 0
Outputs are produced as [d, q] and transposed back on the host.

Scheduling: the PE must never idle (TRN2 DVFS drops the PE clock from
2.4 GHz to 1.2/0.65 GHz after any idle; full speed needs 3us of
continuous execution).  Scores run through 512-wide single-bank PSUM
windows (bufs=2) so exp1(w) overlaps scores(w+1); r1 matmuls trail
their exp1 by 2 windows; the previous blocks' PV/r2 matmul quanta are
interleaved between windows as PE filler.  4-deep block pipeline:
A(n) scores/exp1/r1 | M(n-1) p1-muls | X(n-2) exp2+sub | D(n-3) PV.
exp2 pieces are injected mid-way into the Act window stream so neither
exp1 nor exp2 ever head-blocks the other.

Sharding: data-parallel over batch, B=16 -> 2 per core on 8 cores.
"""

import numpy as np
import ml_dtypes

import concourse.bass as bass
import concourse.mybir as mybir
import concourse.tile as tile
from concourse import bacc
from concourse.bass_utils import run_bass_kernel_spmd

B, S, D, H = 16, 1024, 512, 8
DK = D // H           # 64
NCORES = 8
BLOC = B // NCORES    # 2 batches per core
NCH = S // 128        # 8 k-chunks of 128
F32 = mybir.dt.float32
BF16 = mybir.dt.bfloat16
NPBF16 = ml_dtypes.bfloat16

LIVE = [S - 128 * c for c in range(NCH)]          # live width per chunk
OFF = [sum(LIVE[:c]) for c in range(NCH)]         # packed offset per chunk
PACK = OFF[-1] + LIVE[-1]                         # 4608
NW = PACK // 512                                  # 9 scores windows of 512
# window w covers packed cols [512w, 512w+512): list of (chunk, a, b) with
# a/b chunk-live-local
WPIECES = []
for _w in range(NW):
    _s0, _s1 = 512 * _w, 512 * (_w + 1)
    _ps = []
    for _c in range(NCH):
        _a, _b = max(_s0, OFF[_c]), min(_s1, OFF[_c] + LIVE[_c])
        if _a < _b:
            _ps.append((_c, _a - OFF[_c], _b - OFF[_c]))
    WPIECES.append(_ps)

# number of v parts in the PV sweep: 2 = hi+lo (accurate), 1 = hi only (fast)
VSPLIT = 1
# chunks < XCH get exact exp2 (expm1(x)~=x elsewhere; exact where p1 large)
XCH = 2
# chunks whose mul runs f32 on DVE then converts to e2 (bf16-out
# tensor_tensor on DVE is slow; f32-out mul + tensor_copy is faster)
CONV = {2}
# exp2/sub splits (packed col ranges, within the exact region) and the
# window after which each is emitted into the Act stream
XSPLITS = [(0, 960), (960, 1920)]
XPOS = {3: 0, 6: 1}
# D-quanta drained before w0 and after each window
PRE_DRAIN = 4
DRAINS = [2, 2, 2, 2, 2, 2, 2, 2, 2]

# knobs that test.py can flip
TRACE = False
LAST_RESULTS = None


def build_nc(debug=False):
    nc = bacc.Bacc("TRN2", target_bir_lowering=False, debug=debug)
    AF = mybir.ActivationFunctionType
    ALU = mybir.AluOpType

    qt_d = nc.dram_tensor("qt", [BLOC, H, 128, S], BF16, kind="ExternalInput")
    kt_d = nc.dram_tensor("kt", [BLOC, H, 128, S], BF16, kind="ExternalInput")
    # (1-cm)*[v1|v2] per (b, h, chunk), split into bf16 hi + lo parts
    vcat_d = nc.dram_tensor(
        "vcat", [BLOC, H, NCH, 2, 128, 128], BF16, kind="ExternalInput"
    )
    vtot_d = nc.dram_tensor("vtot", [BLOC, H, 128], F32, kind="ExternalInput")
    # (1-cm) replicated across columns, per (b, chunk): r2 matmul weights
    cmrep_d = nc.dram_tensor("cmrep", [BLOC, NCH, 128, 128], BF16, kind="ExternalInput")
    ind_d = nc.dram_tensor("ind", [128, S], BF16, kind="ExternalInput")
    cnt_d = nc.dram_tensor("cnt", [128, 128], BF16, kind="ExternalInput")
    dmask_d = nc.dram_tensor("dmask", [128, 128], BF16, kind="ExternalInput")
    ident_d = nc.dram_tensor("ident", [128, 128], BF16, kind="ExternalInput")
    ones_d = nc.dram_tensor("onesd", [128, 128], BF16, kind="ExternalInput")
    out1_d = nc.dram_tensor("out1t", [BLOC, D, S], F32, kind="ExternalOutput")
    out2_d = nc.dram_tensor("out2t", [BLOC, D, S], F32, kind="ExternalOutput")

    def bank_pieces(p0, p1):
        """split [p0, p1) at 512-aligned psum bank boundaries"""
        out = []
        p = p0
        while p < p1:
            end = min(p1, (p // 512 + 1) * 512)
            out.append((p, end))
            p = end
        return out

    with tile.TileContext(nc) as tc:
        with (
            tc.tile_pool(name="consts", bufs=1) as consts,
            tc.tile_pool(name="qk", bufs=3) as qkp,
            tc.tile_pool(name="vc", bufs=5) as vcp,
            tc.tile_pool(name="e1", bufs=3) as e1p,
            tc.tile_pool(name="e2", bufs=3) as e2p,
            tc.tile_pool(name="tmp", bufs=2) as tmpp,
            tc.tile_pool(name="rc", bufs=3) as rcp,
            tc.tile_pool(name="outs", bufs=2) as outp,
            tc.tile_pool(name="sc_ps", bufs=2, space="PSUM") as sc_psp,
            tc.tile_pool(name="r_ps", bufs=1, space="PSUM") as r_psp,
            tc.tile_pool(name="o_ps", bufs=1, space="PSUM") as o_psp,
        ):
            vtot_sb = consts.tile([128, BLOC * H], F32)
            nc.sync.dma_start(out=vtot_sb, in_=vtot_d[:].rearrange("b h d -> d (b h)"))
            dm_sb = consts.tile([128, 128], BF16)
            nc.sync.dma_start(out=dm_sb, in_=dmask_d[:, :])
            id_sb = consts.tile([128, 128], BF16)
            nc.sync.dma_start(out=id_sb, in_=ident_d[:, :])
            ones_sb = consts.tile([128, 128], BF16)
            nc.sync.dma_start(out=ones_sb, in_=ones_d[:, :])
            ind_sb = consts.tile([128, S], BF16)
            nc.sync.dma_start(out=ind_sb, in_=ind_d[:, :])
            cnt_sb = consts.tile([128, 128], BF16)
            nc.sync.dma_start(out=cnt_sb, in_=cnt_d[:, :])
            cmrep_sb = consts.tile([128, BLOC, NCH, 128], BF16)
            nc.sync.dma_start(
                out=cmrep_sb, in_=cmrep_d[:].rearrange("b c p j -> p b c j")
            )

            NB = BLOC * H
            st = [dict() for _ in range(NB)]

            def dma_in(blk):
                bi, h = divmod(blk, H)
                s = st[blk]
                qt_sb = qkp.tile([128, S], BF16, tag="qt")
                kt_sb = qkp.tile([128, S], BF16, tag="kt")
                nc.sync.dma_start(out=qt_sb, in_=qt_d[bi, h])
                nc.sync.dma_start(out=kt_sb, in_=kt_d[bi, h])
                vc_sb = vcp.tile([128, NCH, VSPLIT, 128], BF16, tag="vc")
                nc.sync.dma_start(
                    out=vc_sb,
                    in_=vcat_d[bi, h, :, 0:VSPLIT].rearrange("c l p j -> p c l j"),
                )
                s["qt"], s["kt"], s["vc"] = qt_sb, kt_sb, vc_sb

            def emit_sc(blk, w):
                """scores matmuls for packed window w into a 1-bank psum tile"""
                s = st[blk]
                sps = sc_psp.tile([128, 512], F32, tag="sc", name="sps")
                s["sps"][w] = sps
                base = 512 * w
                for c, a, b in WPIECES[w]:
                    q0 = 128 * c
                    loc = OFF[c] + a - base
                    nc.tensor.matmul(
                        sps[:, loc : loc + (b - a)],
                        lhsT=s["kt"][:, q0 : q0 + 128],
                        rhs=s["qt"][:, q0 + a : q0 + b],
                        start=True,
                        stop=(a > 0),
                        skip_group_check=True,
                    )
                    if a == 0:
                        # causal: += I^T @ dmask adds -1e30 on/above diag
                        nc.tensor.matmul(
                            sps[:, loc : loc + 128],
                            lhsT=id_sb,
                            rhs=dm_sb,
                            start=False,
                            stop=True,
                            skip_group_check=True,
                        )

            def emit_exp1(blk, w):
                s = st[blk]
                if w == 0:
                    s["e1"] = e1p.tile([128, PACK], BF16, tag="e1", name="e1")
                base = 512 * w
                nc.scalar.activation(
                    s["e1"][:, base : base + 512],
                    s["sps"][w][:, 0:512],
                    AF.Exp,
                    scale=0.125,
                )

            def emit_r1(blk, w):
                s = st[blk]
                if w == 0:
                    s["r1ps"] = r_psp.tile([128, S], F32, tag="r1", name="r1ps")
                for c, a, b in WPIECES[w]:
                    q0 = 128 * c
                    for p0, p1 in bank_pieces(q0 + a, q0 + b):
                        nc.tensor.matmul(
                            s["r1ps"][:, p0:p1],
                            lhsT=ones_sb,
                            rhs=s["e1"][:, OFF[c] + p0 - q0 : OFF[c] + p1 - q0],
                            start=(c == 0),
                            stop=(c == NCH - 1),
                            skip_group_check=True,
                        )

            def emit_recip1(blk):
                s = st[blk]
                rec1 = rcp.tile([128, S], F32, tag="rec1")
                nc.vector.reciprocal_approx_fast(out=rec1, in_=s["r1ps"][:, 0:S])
                nc.vector.memset(rec1[:, 0:1], 0.0)
                s["rec1"] = rec1

            def phase_M(blk):
                """p1 = e1 * rec1: chunks < XCH -> tmp f32 (exact exp2
                follows); CONV chunks -> tmp f32 then copy to e2 (bf16-out
                tensor_tensor is slow on DVE); others -> e2 on GpSimd"""
                s = st[blk]
                tmax = max([XCH - 1] + list(CONV))
                tmp = tmpp.tile([128, OFF[tmax] + LIVE[tmax]], F32, tag="tmp")
                s["tmp"] = tmp
                s["e2"] = e2p.tile([128, PACK], BF16, tag="e2", name="e2")
                for c in range(NCH):
                    q0 = 128 * c
                    sl = slice(OFF[c], OFF[c] + LIVE[c])
                    if c < XCH or c in CONV:
                        nc.vector.tensor_mul(
                            tmp[:, sl], s["e1"][:, sl], s["rec1"][:, q0:S]
                        )
                        if c in CONV:
                            nc.vector.tensor_copy(out=s["e2"][:, sl], in_=tmp[:, sl])
                    else:
                        nc.gpsimd.tensor_mul(
                            s["e2"][:, sl], s["e1"][:, sl], s["rec1"][:, q0:S]
                        )

            def emit_X(blk, piece):
                """exact exp2 + sub for one chunk-0 split -> e2 bf16"""
                s = st[blk]
                x0, x1 = XSPLITS[piece]
                nc.scalar.activation(s["tmp"][:, x0:x1], s["tmp"][:, x0:x1], AF.Exp)
                nc.vector.tensor_scalar_add(
                    s["e2"][:, x0:x1], s["tmp"][:, x0:x1], -1.0
                )

            def d_quanta(blk):
                """PE-filler quanta for the PV/r2 sweep of an older block"""
                bi, h = divmod(blk, H)
                s = st[blk]
                qs = []

                def q_alloc():
                    s["r2ps"] = r_psp.tile([128, S], F32, tag="r2", name="r2ps")
                    for p0, p1 in bank_pieces(0, S):
                        nc.tensor.matmul(
                            s["r2ps"][:, p0:p1],
                            lhsT=cnt_sb,
                            rhs=ind_sb[:, p0:p1],
                            start=True,
                            stop=False,
                            skip_group_check=True,
                        )

                def q_ot_alloc():
                    s["otps"] = o_psp.tile([128, S], F32, tag="ot", name="otps")

                qs.append(q_ot_alloc)
                for c in range(NCH):
                    q0 = 128 * c
                    for p0, p1 in bank_pieces(q0, S):

                        def q_pv(c=c, q0=q0, p0=p0, p1=p1):
                            ee = s["e2"][:, OFF[c] + p0 - q0 : OFF[c] + p1 - q0]
                            for hl in range(VSPLIT):
                                nc.tensor.matmul(
                                    s["otps"][:, p0:p1],
                                    lhsT=s["vc"][:, c, hl, :],
                                    rhs=ee,
                                    start=(c == 0 and hl == 0),
                                    stop=(c == NCH - 1 and hl == VSPLIT - 1),
                                    skip_group_check=True,
                                )

                        qs.append(q_pv)
                qs.append(q_alloc)
                for c0_ in range(0, NCH, 2):

                    def q_r2(c0_=c0_):
                        for c in (c0_, c0_ + 1):
                            q0 = 128 * c
                            for p0, p1 in bank_pieces(q0, S):
                                nc.tensor.matmul(
                                    s["r2ps"][:, p0:p1],
                                    lhsT=cmrep_sb[:, bi, c, :],
                                    rhs=s["e2"][
                                        :, OFF[c] + p0 - q0 : OFF[c] + p1 - q0
                                    ],
                                    start=False,
                                    stop=(c == NCH - 1),
                                    skip_group_check=True,
                                )

                    qs.append(q_r2)

                def q_fin():
                    rec2 = rcp.tile([128, S], F32, tag="rec2")
                    nc.vector.reciprocal_approx_fast(out=rec2, in_=s["r2ps"][:, 0:S])
                    ot_sb = outp.tile([128, S], F32, tag="otsb")
                    nc.vector.scalar_tensor_tensor(
                        out=ot_sb,
                        in0=s["otps"][:, 0:S],
                        scalar=vtot_sb[:, blk : blk + 1],
                        in1=rec2,
                        op0=ALU.add,
                        op1=ALU.mult,
                    )
                    nc.vector.memset(ot_sb[:, 0:1], 0.0)
                    nc.sync.dma_start(
                        out=out1_d[bi, DK * h : DK * (h + 1), :], in_=ot_sb[0:DK, :]
                    )
                    nc.sync.dma_start(
                        out=out2_d[bi, DK * h : DK * (h + 1), :],
                        in_=ot_sb[DK : 2 * DK, :],
                    )

                qs.append(q_fin)
                return qs

            # 4-deep pipeline: A(n) | M(n-1) | X(n-2) | D(n-3), with D's
            # matmul quanta interleaved into A's window stream as PE filler
            dma_in(0)
            for i in range(NB + 3):
                dq = d_quanta(i - 3) if 3 <= i < NB + 3 else []
                di = 0

                def drain(k):
                    nonlocal di
                    n = min(k, len(dq) - di)
                    for _ in range(n):
                        dq[di]()
                        di += 1

                if i < NB:
                    if i + 1 < NB:
                        dma_in(i + 1)
                    st[i]["sps"] = {}
                    drain(PRE_DRAIN)
                    for w in range(NW):
                        emit_sc(i, w)
                        emit_exp1(i, w)
                        if i >= 2 and w in XPOS:
                            emit_X(i - 2, XPOS[w])
                        drain(DRAINS[w])
                        if w >= 2:
                            emit_r1(i, w - 2)
                    emit_r1(i, NW - 2)
                    emit_r1(i, NW - 1)
                    drain(len(dq))
                    emit_recip1(i)
                else:
                    if i - 2 < NB:
                        for p in range(len(XSPLITS)):
                            emit_X(i - 2, p)
                    drain(len(dq))
                if 1 <= i <= NB:
                    phase_M(i - 1)

    nc.compile()
    return nc


_NC_CACHE = None


def _get_nc():
    global _NC_CACHE
    if _NC_CACHE is None:
        _NC_CACHE = build_nc()
    return _NC_CACHE


def make_in_maps(q, k, v1, v2, cm):
    """Full inputs -> per-core input maps (host-side sharding + layout)."""
    q = np.asarray(q, dtype=np.float32).astype(NPBF16)
    k = np.asarray(k, dtype=np.float32).astype(NPBF16)
    v1 = np.asarray(v1, dtype=np.float32)
    v2 = np.asarray(v2, dtype=np.float32)
    cm = np.asarray(cm)

    # additive causal mask for the diagonal block: 0 where k < q else -1e30
    dmask = np.where(
        np.arange(128)[:, None] < np.arange(128)[None, :], 0.0, -1e30
    ).astype(NPBF16)
    ident = np.eye(128, dtype=NPBF16)
    onesd = np.ones((128, 128), NPBF16)
    ind = np.ones((128, S), np.float32).astype(NPBF16)
    cnt = np.full((128, 128), float(S) / 128.0, np.float32).astype(NPBF16)

    in_maps = []
    for core in range(NCORES):
        b0 = core * BLOC
        qt = np.zeros((BLOC, H, 128, S), NPBF16)  # [b, h, dk(pad 128), s]
        qt[:, :, 0:DK] = q[b0 : b0 + BLOC].reshape(BLOC, S, H, DK).transpose(0, 2, 3, 1)
        kt = np.zeros((BLOC, H, 128, S), NPBF16)
        kt[:, :, 0:DK] = k[b0 : b0 + BLOC].reshape(BLOC, S, H, DK).transpose(0, 2, 3, 1)
        cml = 1.0 - cm[b0 : b0 + BLOC].astype(np.float32)  # [b, s] (1-cm)
        v1s = v1[b0 : b0 + BLOC].reshape(BLOC, NCH, 128, H, DK).transpose(0, 3, 1, 2, 4)
        v2s = v2[b0 : b0 + BLOC].reshape(BLOC, NCH, 128, H, DK).transpose(0, 3, 1, 2, 4)
        vc = np.empty((BLOC, H, NCH, 128, 128), np.float32)
        vc[..., 0:DK] = v1s
        vc[..., DK : 2 * DK] = v2s
        # vtot: unmasked total column sums (the "+1" of every key)
        vtot = np.ascontiguousarray(
            vc.astype(np.float64).sum(axis=(2, 3)).astype(np.float32)
        )  # [b,h,128]
        # counter-mask folded into the PV weights
        vcm = vc * cml.reshape(BLOC, 1, NCH, 128, 1)
        vhi = vcm.astype(NPBF16)
        vlo = (vcm - vhi.astype(np.float32)).astype(NPBF16)
        vcat = np.ascontiguousarray(np.stack([vhi, vlo], axis=3))
        cmrep = np.ascontiguousarray(
            np.broadcast_to(
                cml.reshape(BLOC, NCH, 128, 1), (BLOC, NCH, 128, 128)
            ).astype(NPBF16)
        )
        in_maps.append(
            dict(
                qt=qt, kt=kt, vcat=vcat, vtot=vtot, cmrep=cmrep,
                ind=ind, cnt=cnt, dmask=dmask, ident=ident, onesd=onesd,
            )
        )
    return in_maps


def _gather(res):
    out1 = np.concatenate(
        [r["out1t"].transpose(0, 2, 1) for r in res.results], axis=0
    )
    out2 = np.concatenate(
        [r["out2t"].transpose(0, 2, 1) for r in res.results], axis=0
    )
    return np.ascontiguousarray(out1), np.ascontiguousarray(out2)


def kernel(q, k, v1, v2, counter_attention_mask):
    global LAST_RESULTS
    in_maps = make_in_maps(q, k, v1, v2, counter_attention_mask)
    nc = _get_nc()
    res = run_bass_kernel_spmd(
        nc, in_maps, core_ids=list(range(NCORES)), trace=TRACE
    )
    LAST_RESULTS = res
    return _gather(res)


# revision 8
# speedup vs baseline: 1.1968x; 1.1968x over previous
"""Trainium2 Bass kernel for dual-attention (DisKT-style) nn module.

Math per (batch, head) with S=1024, dk=64, all on-chip in [k, q] layout:
    sT       = (k_h @ q_h^T)            (+ -1e30 on causal-dead diag block)
    E1T      = exp(sT / 8)              (causally-dead region never computed)
    r1[q]    = sum_k E1T[k, q]          (ones^T @ E1T, PSUM broadcast rows)
    p1       = E1T * rec1[q]
    E2''     = exp(p1) - 1              <- bf16-friendly: small values keep
                                           precision; the "+1" of every key
                                           becomes an exact rank-1 vtot fixup
    outT     = (cm*vhi)^T @ E2'' + (cm*vlo)^T @ E2''   (cm pre-masked on host,
                                            v split hi+lo kills bf16 v error)
    r2       = 1024 + cmrep^T @ E2''
    out      = (outT + vtot) * (1/r2) ;  out[:, q=0] = 0
Outputs are produced as [d, q] and transposed back on the host.

Scheduling: the PE must never idle (TRN2 DVFS drops the PE clock from
2.4 GHz to 1.2/0.65 GHz after any idle; full speed needs 3us of
continuous execution).  Scores run through 512-wide single-bank PSUM
windows (bufs=2) so exp1(w) overlaps scores(w+1); r1 matmuls trail
their exp1 by 2 windows; the previous blocks' PV/r2 matmul quanta are
interleaved between windows as PE filler.  4-deep block pipeline:
A(n) scores/exp1/r1 | M(n-1) p1-muls | X(n-2) exp2+sub | D(n-3) PV.
exp2 pieces are injected mid-way into the Act window stream so neither
exp1 nor exp2 ever head-blocks the other.

Sharding: data-parallel over batch, B=16 -> 2 per core on 8 cores.
"""

import numpy as np
import ml_dtypes

import concourse.bass as bass
import concourse.mybir as mybir
import concourse.tile as tile
from concourse import bacc
from concourse.bass_utils import run_bass_kernel_spmd

B, S, D, H = 16, 1024, 512, 8
DK = D // H           # 64
NCORES = 8
BLOC = B // NCORES    # 2 batches per core
NCH = S // 128        # 8 k-chunks of 128
F32 = mybir.dt.float32
BF16 = mybir.dt.bfloat16
NPBF16 = ml_dtypes.bfloat16

LIVE = [S - 128 * c for c in range(NCH)]          # live width per chunk
OFF = [sum(LIVE[:c]) for c in range(NCH)]         # packed offset per chunk
PACK = OFF[-1] + LIVE[-1]                         # 4608
NW = PACK // 512                                  # 9 scores windows of 512
# window w covers packed cols [512w, 512w+512): list of (chunk, a, b) with
# a/b chunk-live-local
WPIECES = []
for _w in range(NW):
    _s0, _s1 = 512 * _w, 512 * (_w + 1)
    _ps = []
    for _c in range(NCH):
        _a, _b = max(_s0, OFF[_c]), min(_s1, OFF[_c] + LIVE[_c])
        if _a < _b:
            _ps.append((_c, _a - OFF[_c], _b - OFF[_c]))
    WPIECES.append(_ps)

# number of v parts in the PV sweep: 2 = hi+lo (accurate), 1 = hi only (fast)
VSPLIT = 1
# chunks < XCH get exact exp2 (expm1(x)~=x elsewhere; exact where p1 large)
XCH = 1
# chunks whose mul runs f32 on DVE then converts to e2 via tensor_copy
CONV = set()
# chunks whose (direct bf16) mul runs on DVE rather than GpSimd
MUL_ON_DVE = {1}
# exp2/sub splits (packed col ranges, within the exact region) and the
# window after which each is emitted into the Act stream
XSPLITS = [(0, 512), (512, 1024)]
XPOS = {3: 0, 6: 1}
# D-quanta drained before w0 and after each window
PRE_DRAIN = 2
DRAINS = [2, 2, 2, 2, 2, 2, 2, 3, 3]

# knobs that test.py can flip
TRACE = False
LAST_RESULTS = None


def build_nc(debug=False):
    nc = bacc.Bacc("TRN2", target_bir_lowering=False, debug=debug)
    AF = mybir.ActivationFunctionType
    ALU = mybir.AluOpType

    qt_d = nc.dram_tensor("qt", [BLOC, H, 128, S], BF16, kind="ExternalInput")
    kt_d = nc.dram_tensor("kt", [BLOC, H, 128, S], BF16, kind="ExternalInput")
    # (1-cm)*[v1|v2] per (b, h, chunk), split into bf16 hi + lo parts
    vcat_d = nc.dram_tensor(
        "vcat", [BLOC, H, NCH, 2, 128, 128], BF16, kind="ExternalInput"
    )
    vtot_d = nc.dram_tensor("vtot", [BLOC, H, 128], F32, kind="ExternalInput")
    # (1-cm) replicated across columns, per (b, chunk): r2 matmul weights
    cmrep_d = nc.dram_tensor("cmrep", [BLOC, NCH, 128, 128], BF16, kind="ExternalInput")
    ind_d = nc.dram_tensor("ind", [128, S], BF16, kind="ExternalInput")
    cnt_d = nc.dram_tensor("cnt", [128, 128], BF16, kind="ExternalInput")
    dmask_d = nc.dram_tensor("dmask", [128, 128], BF16, kind="ExternalInput")
    ident_d = nc.dram_tensor("ident", [128, 128], BF16, kind="ExternalInput")
    ones_d = nc.dram_tensor("onesd", [128, 128], BF16, kind="ExternalInput")
    out1_d = nc.dram_tensor("out1t", [BLOC, D, S], F32, kind="ExternalOutput")
    out2_d = nc.dram_tensor("out2t", [BLOC, D, S], F32, kind="ExternalOutput")

    def bank_pieces(p0, p1):
        """split [p0, p1) at 512-aligned psum bank boundaries"""
        out = []
        p = p0
        while p < p1:
            end = min(p1, (p // 512 + 1) * 512)
            out.append((p, end))
            p = end
        return out

    with tile.TileContext(nc) as tc:
        with (
            tc.tile_pool(name="consts", bufs=1) as consts,
            tc.tile_pool(name="qk", bufs=3) as qkp,
            tc.tile_pool(name="vc", bufs=5) as vcp,
            tc.tile_pool(name="e1", bufs=3) as e1p,
            tc.tile_pool(name="e2", bufs=3) as e2p,
            tc.tile_pool(name="tmp", bufs=2) as tmpp,
            tc.tile_pool(name="rc", bufs=3) as rcp,
            tc.tile_pool(name="outs", bufs=2) as outp,
            tc.tile_pool(name="sc_ps", bufs=2, space="PSUM") as sc_psp,
            tc.tile_pool(name="r_ps", bufs=1, space="PSUM") as r_psp,
            tc.tile_pool(name="o_ps", bufs=1, space="PSUM") as o_psp,
        ):
            vtot_sb = consts.tile([128, BLOC * H], F32)
            nc.sync.dma_start(out=vtot_sb, in_=vtot_d[:].rearrange("b h d -> d (b h)"))
            dm_sb = consts.tile([128, 128], BF16)
            nc.sync.dma_start(out=dm_sb, in_=dmask_d[:, :])
            id_sb = consts.tile([128, 128], BF16)
            nc.sync.dma_start(out=id_sb, in_=ident_d[:, :])
            ones_sb = consts.tile([128, 128], BF16)
            nc.sync.dma_start(out=ones_sb, in_=ones_d[:, :])
            ind_sb = consts.tile([128, S], BF16)
            nc.sync.dma_start(out=ind_sb, in_=ind_d[:, :])
            cnt_sb = consts.tile([128, 128], BF16)
            nc.sync.dma_start(out=cnt_sb, in_=cnt_d[:, :])
            cmrep_sb = consts.tile([128, BLOC, NCH, 128], BF16)
            nc.sync.dma_start(
                out=cmrep_sb, in_=cmrep_d[:].rearrange("b c p j -> p b c j")
            )

            NB = BLOC * H
            st = [dict() for _ in range(NB)]

            def dma_in(blk):
                bi, h = divmod(blk, H)
                s = st[blk]
                qt_sb = qkp.tile([128, S], BF16, tag="qt")
                kt_sb = qkp.tile([128, S], BF16, tag="kt")
                nc.sync.dma_start(out=qt_sb, in_=qt_d[bi, h])
                nc.sync.dma_start(out=kt_sb, in_=kt_d[bi, h])
                vc_sb = vcp.tile([128, NCH, VSPLIT, 128], BF16, tag="vc")
                nc.sync.dma_start(
                    out=vc_sb,
                    in_=vcat_d[bi, h, :, 0:VSPLIT].rearrange("c l p j -> p c l j"),
                )
                s["qt"], s["kt"], s["vc"] = qt_sb, kt_sb, vc_sb

            def emit_sc(blk, w):
                """scores matmuls for packed window w into a 1-bank psum tile"""
                s = st[blk]
                sps = sc_psp.tile([128, 512], F32, tag="sc", name="sps")
                s["sps"][w] = sps
                base = 512 * w
                for c, a, b in WPIECES[w]:
                    q0 = 128 * c
                    loc = OFF[c] + a - base
                    nc.tensor.matmul(
                        sps[:, loc : loc + (b - a)],
                        lhsT=s["kt"][:, q0 : q0 + 128],
                        rhs=s["qt"][:, q0 + a : q0 + b],
                        start=True,
                        stop=(a > 0),
                        skip_group_check=True,
                    )
                    if a == 0:
                        # causal: += I^T @ dmask adds -1e30 on/above diag
                        nc.tensor.matmul(
                            sps[:, loc : loc + 128],
                            lhsT=id_sb,
                            rhs=dm_sb,
                            start=False,
                            stop=True,
                            skip_group_check=True,
                        )

            def emit_exp1(blk, w):
                s = st[blk]
                if w == 0:
                    s["e1"] = e1p.tile([128, PACK], BF16, tag="e1", name="e1")
                base = 512 * w
                nc.scalar.activation(
                    s["e1"][:, base : base + 512],
                    s["sps"][w][:, 0:512],
                    AF.Exp,
                    scale=0.125,
                )

            def emit_r1(blk, w):
                s = st[blk]
                if w == 0:
                    s["r1ps"] = r_psp.tile([128, S], F32, tag="r1", name="r1ps")
                for c, a, b in WPIECES[w]:
                    q0 = 128 * c
                    for p0, p1 in bank_pieces(q0 + a, q0 + b):
                        nc.tensor.matmul(
                            s["r1ps"][:, p0:p1],
                            lhsT=ones_sb,
                            rhs=s["e1"][:, OFF[c] + p0 - q0 : OFF[c] + p1 - q0],
                            start=(c == 0),
                            stop=(c == NCH - 1),
                            skip_group_check=True,
                        )

            def emit_recip1(blk):
                s = st[blk]
                rec1 = rcp.tile([128, S], F32, tag="rec1")
                nc.vector.reciprocal_approx_fast(out=rec1, in_=s["r1ps"][:, 0:S])
                nc.vector.memset(rec1[:, 0:1], 0.0)
                s["rec1"] = rec1

            def phase_M(blk):
                """p1 = e1 * rec1: chunks < XCH -> tmp f32 (exact exp2
                follows); CONV chunks -> tmp f32 then copy to e2 (bf16-out
                tensor_tensor is slow on DVE); others -> e2 on GpSimd"""
                s = st[blk]
                tmax = max([XCH - 1] + list(CONV))
                tmp = tmpp.tile([128, OFF[tmax] + LIVE[tmax]], F32, tag="tmp")
                s["tmp"] = tmp
                s["e2"] = e2p.tile([128, PACK], BF16, tag="e2", name="e2")
                for c in range(NCH):
                    q0 = 128 * c
                    sl = slice(OFF[c], OFF[c] + LIVE[c])
                    if c < XCH or c in CONV:
                        nc.vector.tensor_mul(
                            tmp[:, sl], s["e1"][:, sl], s["rec1"][:, q0:S]
                        )
                        if c in CONV:
                            nc.vector.tensor_copy(out=s["e2"][:, sl], in_=tmp[:, sl])
                    else:
                        eng = nc.vector if c in MUL_ON_DVE else nc.gpsimd
                        eng.tensor_mul(
                            s["e2"][:, sl], s["e1"][:, sl], s["rec1"][:, q0:S]
                        )

            def emit_X(blk, piece):
                """exact exp2 + sub for one chunk-0 split -> e2 bf16"""
                s = st[blk]
                x0, x1 = XSPLITS[piece]
                nc.scalar.activation(s["tmp"][:, x0:x1], s["tmp"][:, x0:x1], AF.Exp)
                nc.vector.tensor_scalar_add(
                    s["e2"][:, x0:x1], s["tmp"][:, x0:x1], -1.0
                )

            def d_quanta(blk):
                """PE-filler quanta for the PV/r2 sweep of an older block"""
                bi, h = divmod(blk, H)
                s = st[blk]
                qs = []

                def q_alloc():
                    s["r2ps"] = r_psp.tile([128, S], F32, tag="r2", name="r2ps")
                    for p0, p1 in bank_pieces(0, S):
                        nc.tensor.matmul(
                            s["r2ps"][:, p0:p1],
                            lhsT=cnt_sb,
                            rhs=ind_sb[:, p0:p1],
                            start=True,
                            stop=False,
                            skip_group_check=True,
                        )

                def q_ot_alloc():
                    s["otps"] = o_psp.tile([128, S], F32, tag="ot", name="otps")

                qs.append(q_ot_alloc)
                for c in range(NCH):
                    q0 = 128 * c
                    for p0, p1 in bank_pieces(q0, S):

                        def q_pv(c=c, q0=q0, p0=p0, p1=p1):
                            ee = s["e2"][:, OFF[c] + p0 - q0 : OFF[c] + p1 - q0]
                            for hl in range(VSPLIT):
                                nc.tensor.matmul(
                                    s["otps"][:, p0:p1],
                                    lhsT=s["vc"][:, c, hl, :],
                                    rhs=ee,
                                    start=(c == 0 and hl == 0),
                                    stop=(c == NCH - 1 and hl == VSPLIT - 1),
                                    skip_group_check=True,
                                )

                        qs.append(q_pv)
                qs.append(q_alloc)
                for cs_ in ([0], [1], [2], [3], [4, 5], [6, 7]):

                    def q_r2(cs_=cs_):
                        for c in cs_:
                            q0 = 128 * c
                            for p0, p1 in bank_pieces(q0, S):
                                nc.tensor.matmul(
                                    s["r2ps"][:, p0:p1],
                                    lhsT=cmrep_sb[:, bi, c, :],
                                    rhs=s["e2"][
                                        :, OFF[c] + p0 - q0 : OFF[c] + p1 - q0
                                    ],
                                    start=False,
                                    stop=(c == NCH - 1),
                                    skip_group_check=True,
                                )

                    qs.append(q_r2)

                def q_fin():
                    rec2 = rcp.tile([128, S], F32, tag="rec2")
                    nc.vector.reciprocal_approx_fast(out=rec2, in_=s["r2ps"][:, 0:S])
                    ot_sb = outp.tile([128, S], F32, tag="otsb")
                    nc.vector.scalar_tensor_tensor(
                        out=ot_sb,
                        in0=s["otps"][:, 0:S],
                        scalar=vtot_sb[:, blk : blk + 1],
                        in1=rec2,
                        op0=ALU.add,
                        op1=ALU.mult,
                    )
                    nc.vector.memset(ot_sb[:, 0:1], 0.0)
                    nc.sync.dma_start(
                        out=out1_d[bi, DK * h : DK * (h + 1), :], in_=ot_sb[0:DK, :]
                    )
                    nc.sync.dma_start(
                        out=out2_d[bi, DK * h : DK * (h + 1), :],
                        in_=ot_sb[DK : 2 * DK, :],
                    )

                qs.append(q_fin)
                return qs

            # 4-deep pipeline: A(n) | M(n-1) | X(n-2) | D(n-3), with D's
            # matmul quanta interleaved into A's window stream as PE filler
            dma_in(0)
            for i in range(NB + 3):
                dq = d_quanta(i - 3) if 3 <= i < NB + 3 else []
                di = 0

                def drain(k):
                    nonlocal di
                    n = min(k, len(dq) - di)
                    for _ in range(n):
                        dq[di]()
                        di += 1

                if i < NB:
                    if i + 1 < NB:
                        dma_in(i + 1)
                    st[i]["sps"] = {}
                    drain(PRE_DRAIN)
                    for w in range(NW):
                        emit_sc(i, w)
                        emit_exp1(i, w)
                        if i >= 2 and w in XPOS:
                            emit_X(i - 2, XPOS[w])
                        drain(DRAINS[w])
                        if w >= 2:
                            emit_r1(i, w - 2)
                    emit_r1(i, NW - 2)
                    emit_r1(i, NW - 1)
                    drain(len(dq))
                    emit_recip1(i)
                else:
                    if i - 2 < NB:
                        for p in range(len(XSPLITS)):
                            emit_X(i - 2, p)
                    drain(len(dq))
                if 1 <= i <= NB:
                    phase_M(i - 1)

    nc.compile()
    return nc


_NC_CACHE = None


def _get_nc():
    global _NC_CACHE
    if _NC_CACHE is None:
        _NC_CACHE = build_nc()
    return _NC_CACHE


def make_in_maps(q, k, v1, v2, cm):
    """Full inputs -> per-core input maps (host-side sharding + layout)."""
    q = np.asarray(q, dtype=np.float32).astype(NPBF16)
    k = np.asarray(k, dtype=np.float32).astype(NPBF16)
    v1 = np.asarray(v1, dtype=np.float32)
    v2 = np.asarray(v2, dtype=np.float32)
    cm = np.asarray(cm)

    # additive causal mask for the diagonal block: 0 where k < q else -1e30
    dmask = np.where(
        np.arange(128)[:, None] < np.arange(128)[None, :], 0.0, -1e30
    ).astype(NPBF16)
    ident = np.eye(128, dtype=NPBF16)
    onesd = np.ones((128, 128), NPBF16)
    ind = np.ones((128, S), np.float32).astype(NPBF16)
    cnt = np.full((128, 128), float(S) / 128.0, np.float32).astype(NPBF16)

    in_maps = []
    for core in range(NCORES):
        b0 = core * BLOC
        qt = np.zeros((BLOC, H, 128, S), NPBF16)  # [b, h, dk(pad 128), s]
        qt[:, :, 0:DK] = q[b0 : b0 + BLOC].reshape(BLOC, S, H, DK).transpose(0, 2, 3, 1)
        kt = np.zeros((BLOC, H, 128, S), NPBF16)
        kt[:, :, 0:DK] = k[b0 : b0 + BLOC].reshape(BLOC, S, H, DK).transpose(0, 2, 3, 1)
        cml = 1.0 - cm[b0 : b0 + BLOC].astype(np.float32)  # [b, s] (1-cm)
        v1s = v1[b0 : b0 + BLOC].reshape(BLOC, NCH, 128, H, DK).transpose(0, 3, 1, 2, 4)
        v2s = v2[b0 : b0 + BLOC].reshape(BLOC, NCH, 128, H, DK).transpose(0, 3, 1, 2, 4)
        vc = np.empty((BLOC, H, NCH, 128, 128), np.float32)
        vc[..., 0:DK] = v1s
        vc[..., DK : 2 * DK] = v2s
        # vtot: unmasked total column sums (the "+1" of every key)
        vtot = np.ascontiguousarray(
            vc.astype(np.float64).sum(axis=(2, 3)).astype(np.float32)
        )  # [b,h,128]
        # counter-mask folded into the PV weights
        vcm = vc * cml.reshape(BLOC, 1, NCH, 128, 1)
        vhi = vcm.astype(NPBF16)
        vlo = (vcm - vhi.astype(np.float32)).astype(NPBF16)
        vcat = np.ascontiguousarray(np.stack([vhi, vlo], axis=3))
        cmrep = np.ascontiguousarray(
            np.broadcast_to(
                cml.reshape(BLOC, NCH, 128, 1), (BLOC, NCH, 128, 128)
            ).astype(NPBF16)
        )
        in_maps.append(
            dict(
                qt=qt, kt=kt, vcat=vcat, vtot=vtot, cmrep=cmrep,
                ind=ind, cnt=cnt, dmask=dmask, ident=ident, onesd=onesd,
            )
        )
    return in_maps


def _gather(res):
    out1 = np.concatenate(
        [r["out1t"].transpose(0, 2, 1) for r in res.results], axis=0
    )
    out2 = np.concatenate(
        [r["out2t"].transpose(0, 2, 1) for r in res.results], axis=0
    )
    return np.ascontiguousarray(out1), np.ascontiguousarray(out2)


def kernel(q, k, v1, v2, counter_attention_mask):
    global LAST_RESULTS
    in_maps = make_in_maps(q, k, v1, v2, counter_attention_mask)
    nc = _get_nc()
    res = run_bass_kernel_spmd(
        nc, in_maps, core_ids=list(range(NCORES)), trace=TRACE
    )
    LAST_RESULTS = res
    return _gather(res)
